# revision 14
# baseline (speedup 1.0000x reference)
# Trainium2 Bass kernel for nn_BertAdapter_SLT_49933289783411
#
# Reference computation:
#   y   = tt_linear(x) + bias          (TT-factorized 768->768 linear)
#   out = x + gelu_exact(y)
#
# Key math: the TT cores with ranks [1,5,5,5,5,5,1] factor the 768x768
# weight as W = A @ B with A:(768,5), B:(5,768).  We precompute A,B on
# host (tiny, exact) and run a rank-5 bottleneck matmul on device.
#
# Sharding: data-parallel over the batch dim (8 batch elements -> 8 cores).
# Each core handles x_c:(512,768).  All I/O is bf16 (halves HBM traffic;
# the 2e-2 rel-err budget dwarfs bf16 rounding).  x is pre-transposed on
# host to x^T (feature-major) so the contraction dim lands on SBUF
# partitions.  The 512 rows are processed as 4 quarters of 128 rows, each
# flowing load -> mm1 -> cast -> mm2 -> gelu -> add -> store so the ACT
# engine (the serial bottleneck: ~2.6us of gelu element work) starts as
# early as possible and every stage pipelines across quarters.
#
# Per quarter q (all operands bf16, PSUM accumulation f32):
#   t3_q   = A^T @ x^T_q            (5,128)   PSUM, accumulate over 6 f-chunks
#   y^T_q  = B6^T @ t36_q           (128,768) K=6: B6 rows 0-4 = B, row 5 =
#                                   bias against an all-ones t3 row 5
#   o^T_q  = x^T_q + gelu(y^T_q)    one N=768 gelu op straight from PSUM
#
# B is shipped compact as (6,768) bf16 (9KB) instead of zero-padded to
# K=128 (196KB).  A (128x30 bf16) rides in the head of the x tensor so the
# sync queue issues exactly one DMA per quarter; B goes on the scalar
# (ACT HWDGE) queue; stores alternate sync/gpsimd queues to split the
# ~600ns per-DMA sequencer issue cost.

import numpy as np
import ml_dtypes

import concourse.bass as bass
import concourse.bacc as bacc
import concourse.mybir as mybir
import concourse.tile as tile
from concourse.bass_utils import run_bass_kernel_spmd

HID = 768
ROWS = 512
NPARTS = 4
PSIZE = ROWS // NPARTS      # 128 rows per quarter
NCORES = 8
FCH = 6                     # 768 / 128 feature chunks
RANK = 5
KDIM = RANK + 1             # rank rows + ones row carrying the bias
F32 = mybir.dt.float32
BF16 = mybir.dt.bfloat16

N_WARMUP = 36               # dummy PE matmuls to trip the HAM clock un-throttle

A_COLS = FCH * RANK                        # 30
XT_COLS = A_COLS + NPARTS * HID            # 30 + 3072

_CACHE = {}


class _LeanTileContext(tile.TileContext):
    """TileContext with a minimal exit sequence.

    The stock exit emits drain + all-engine barrier + per-sem clears +
    barrier (~2-3us).  The runtime re-initializes semaphore state on every
    NEFF execution (verified empirically: repeated executions of the same
    loaded executable stay bit-correct without the clears), so only the
    drain — which makes the kernel end wait for the output DMAs — is kept.
    """

    def _drain_and_barrier(self, tick_clock, wait_clock):
        drain_inst = self.nc.sync.drain()
        wait_clock.add_sem_waits(
            drain_inst.ins, tile.ScopedClock({None: tick_clock.global_clock})
        )
        popped = self.nc._tile_sem_poison_stack.pop()
        assert popped is self._sem_poison


def _build_program(act=None):
    if act is None:
        act = mybir.ActivationFunctionType.Gelu
    nc = bacc.Bacc(None, target_bir_lowering=False)
    xt = nc.dram_tensor("xt", [128, XT_COLS], BF16, kind="ExternalInput")
    bm = nc.dram_tensor("bm", [KDIM, HID], BF16, kind="ExternalInput")
    outt = nc.dram_tensor("outt", [128, NPARTS * HID], BF16, kind="ExternalOutput")

    with _LeanTileContext(nc) as tc:
        with (
            tc.tile_pool(name="const", bufs=1) as cpool,
            tc.tile_pool(name="xs", bufs=1) as xpool,
            tc.tile_pool(name="work", bufs=2) as wpool,
            tc.tile_pool(name="ps_t3", bufs=2, space="PSUM") as tpool,
            tc.tile_pool(name="ps_o", bufs=2, space="PSUM") as opool,
            tc.tile_pool(name="ps_w", bufs=1, space="PSUM") as wps_pool,
        ):
            # B lands on the scalar-engine HWDGE queue so the sync queue's
            # serial ~600ns-per-DMA issue budget is spent on x alone
            bm_sb = cpool.tile([KDIM, HID], BF16)
            nc.scalar.dma_start(bm_sb[:], bm[:])

            x_sb = xpool.tile([128, XT_COLS], BF16)
            a_view = x_sb[:, 0:A_COLS]

            def xq(q, c=0):
                return x_sb[:, A_COLS + q * HID + c * PSIZE : A_COLS + q * HID + (c + 1) * PSIZE]

            # q0 (split in two so mm1 q0's ~2.2us DMA-completion-sem wait
            # starts from an earlier, smaller first transfer) and q1 go on
            # the sync HWDGE queue; q2/q3 go on the gpsimd SWDGE queue,
            # issuing and wiring in parallel so the back-half quarters'
            # completion sems arrive ~1us earlier
            cuts = [0, A_COLS + HID // 2] + [A_COLS + q * HID for q in range(1, NPARTS + 1)]
            for i, (s, e) in enumerate(zip(cuts[:-1], cuts[1:])):
                dma = nc.sync if i < 3 else nc.gpsimd
                dma.dma_start(x_sb[:, s:e], xt[:, s:e])

            # PE warmup: garbage matmuls so the HAM clock gate opens while
            # the x loads are still in flight.  The gate needs ~3.4us of
            # sustained PE activity; 26 x 107ns cold matmuls fill the load
            # window so the real matmuls run at 2.4GHz instead of 1.2.
            wsb = cpool.tile([128, 128], BF16)
            nc.gpsimd.memset(wsb[:], 0.0)
            wps = wps_pool.tile([128, 128], F32)
            for _ in range(N_WARMUP):
                nc.tensor.matmul(wps[:], wsb[:], wsb[:], start=True, stop=True)

            # rows 0-4 of t3_sb get the per-quarter TT activations; row 5
            # stays at the memset 1.0 and meets the bias row of bm_sb in mm2
            t3_sb = cpool.tile([128, ROWS], BF16)
            nc.gpsimd.memset(t3_sb[:], 1.0)

            for q in range(NPARTS):
                t3_ps = tpool.tile([RANK, PSIZE], F32, tag="t3_ps")
                for c in range(FCH):
                    nc.tensor.matmul(
                        t3_ps[:],
                        a_view[:, c * RANK : (c + 1) * RANK],
                        xq(q, c),
                        start=(c == 0),
                        stop=(c == FCH - 1),
                    )
                nc.vector.tensor_copy(
                    t3_sb[0:RANK, q * PSIZE : (q + 1) * PSIZE], t3_ps[:]
                )

                # (128,1024) f32 = exactly 2 PSUM banks; cols 0-767 used.
                # start=True on the first matmul touching each bank clears
                # that bank's has_written bits; later ones overwrite their
                # still-clear regions.
                o_ps = opool.tile([128, 1024], F32, tag="o_ps")
                for j in range(FCH):
                    nc.tensor.matmul(
                        o_ps[:, j * PSIZE : (j + 1) * PSIZE],
                        bm_sb[:, j * PSIZE : (j + 1) * PSIZE],
                        t3_sb[0:KDIM, q * PSIZE : (q + 1) * PSIZE],
                        start=(j in (0, 4)),
                        stop=(j in (3, 5)),
                    )
                xq_full = x_sb[:, A_COLS + q * HID : A_COLS + (q + 1) * HID]
                o_sb = wpool.tile([128, HID], BF16, tag="o_sb", bufs=4)
                # one N=768 gelu per quarter straight from PSUM amortizes
                # the ~293ns per-op ACT overhead over the whole quarter
                g_sb = wpool.tile([128, HID], BF16, tag="g_sb", bufs=2)
                nc.scalar.activation(g_sb[:], o_ps[:, 0:HID], act, scale=1.0)
                if q < NPARTS - 1:
                    nc.vector.tensor_add(o_sb[:], g_sb[:], xq_full)
                    dma = nc.sync if q % 2 == 0 else nc.gpsimd
                    dma.dma_start(outt[:, q * HID : (q + 1) * HID], o_sb[:])
                else:
                    # last quarter: add+store in column halves across both
                    # DGE queues so the final ~2.2us store receipt starts
                    # during the second half's add
                    HH = HID // 2
                    for k, dma in ((0, nc.sync), (1, nc.gpsimd)):
                        nc.vector.tensor_add(
                            o_sb[:, k * HH : (k + 1) * HH],
                            g_sb[:, k * HH : (k + 1) * HH],
                            xq_full[:, k * HH : (k + 1) * HH],
                        )
                        dma.dma_start(
                            outt[:, q * HID + k * HH : q * HID + (k + 1) * HH],
                            o_sb[:, k * HH : (k + 1) * HH],
                        )

    nc.finalize()
    return nc


def _get_program():
    if "nc" not in _CACHE:
        _CACHE["nc"] = _build_program()
    return _CACHE["nc"]


def _host_prep(hidden_states, bias, cores):
    """Collapse TT cores to rank-5 factors; pack A + x^T per core in bf16."""
    c0, c1, c2, c3, c4, c5 = [c.astype(np.float64) for c in cores]
    A = np.einsum("iv,vjw,wkx->ijkx", c0[0], c1, c2).reshape(HID, RANK)
    Bm = np.einsum("xpy,yqz,zr->xpqr", c3, c4, c5[:, :, 0]).reshape(RANK, HID)

    a_p = np.ascontiguousarray(
        A.reshape(FCH, 128, RANK).transpose(1, 0, 2).reshape(128, A_COLS)
    ).astype(ml_dtypes.bfloat16)                       # (128, 30)
    bm_p = np.empty((KDIM, HID), dtype=ml_dtypes.bfloat16)
    bm_p[:RANK] = Bm.astype(ml_dtypes.bfloat16)
    bm_p[RANK] = bias.astype(ml_dtypes.bfloat16)       # meets t3_sb's ones row

    xts = []
    for cidx in range(NCORES):
        xct = hidden_states[cidx].T                    # (768, 512) f32
        blocks = [a_p]
        for q in range(NPARTS):
            blocks.append(
                np.ascontiguousarray(xct[:, q * PSIZE : (q + 1) * PSIZE])
                .reshape(FCH, 128, PSIZE)
                .transpose(1, 0, 2)
                .reshape(128, FCH * PSIZE)
                .astype(ml_dtypes.bfloat16)
            )
        xts.append(np.ascontiguousarray(np.concatenate(blocks, axis=1)))
    return xts, bm_p


def _unpack_out(outt_list):
    """outt[p, q*768 + j*128 + r] = out[q*128+r, j*128+p] -> (8, 512, 768)."""
    outs = []
    for outt in outt_list:
        m = np.asarray(outt).reshape(128, NPARTS, FCH, PSIZE)
        o = m.transpose(1, 3, 2, 0).reshape(ROWS, HID)
        outs.append(o)
    return np.stack(outs, axis=0).astype(np.float32)


def run(inputs, trace=False, **spmd_kwargs):
    hidden_states = np.asarray(inputs["hidden_states"], dtype=np.float32)
    bias = np.asarray(inputs["bias"], dtype=np.float32)
    cores = [np.asarray(inputs[f"core{i}"], dtype=np.float32) for i in range(6)]

    xts, bm_p = _host_prep(hidden_states, bias, cores)
    nc = _get_program()
    in_maps = [{"xt": xts[c], "bm": bm_p} for c in range(NCORES)]
    res = run_bass_kernel_spmd(
        nc, in_maps, core_ids=list(range(NCORES)), trace=trace, **spmd_kwargs
    )
    out = _unpack_out([res.results[c]["outt"] for c in range(NCORES)])
    if trace:
        return out, res
    return out


def kernel(**inputs):
    return run(inputs)


# revision 16
# speedup vs baseline: 1.0421x; 1.0421x over previous
# Trainium2 Bass kernel for nn_BertAdapter_SLT_49933289783411
#
# Reference computation:
#   y   = tt_linear(x) + bias          (TT-factorized 768->768 linear)
#   out = x + gelu_exact(y)
#
# Key math: the TT cores with ranks [1,5,5,5,5,5,1] factor the 768x768
# weight as W = A @ B with A:(768,5), B:(5,768).  We precompute A,B on
# host (tiny, exact) and run a rank-5 bottleneck matmul on device.
#
# Sharding: data-parallel over the batch dim (8 batch elements -> 8 cores).
# Each core handles x_c:(512,768).  All I/O is bf16 (halves HBM traffic;
# the 2e-2 rel-err budget dwarfs bf16 rounding).  x is pre-transposed on
# host to x^T (feature-major) so the contraction dim lands on SBUF
# partitions.  The 512 rows are processed as 4 quarters of 128 rows, each
# flowing load -> mm1 -> cast -> mm2 -> gelu -> add -> store so the ACT
# engine (the serial bottleneck: ~2.6us of gelu element work) starts as
# early as possible and every stage pipelines across quarters.
#
# Per quarter q (all operands bf16, PSUM accumulation f32):
#   t3_q   = A^T @ x^T_q            (5,128)   PSUM, accumulate over 6 f-chunks
#   y^T_q  = B6^T @ t36_q           (128,768) K=6: B6 rows 0-4 = B, row 5 =
#                                   bias against an all-ones t3 row 5
#   o^T_q  = x^T_q + gelu(y^T_q)    one N=768 gelu op straight from PSUM
#
# B is shipped compact as (6,768) bf16 (9KB) instead of zero-padded to
# K=128 (196KB).  A (128x30 bf16) rides in the head of the x tensor so the
# sync queue issues exactly one DMA per quarter; B goes on the scalar
# (ACT HWDGE) queue; stores alternate sync/gpsimd queues to split the
# ~600ns per-DMA sequencer issue cost.

import numpy as np
import ml_dtypes

import concourse.bass as bass
import concourse.bacc as bacc
import concourse.mybir as mybir
import concourse.tile as tile
from concourse.bass_utils import run_bass_kernel_spmd

HID = 768
ROWS = 512
NPARTS = 4
PSIZE = ROWS // NPARTS      # 128 rows per quarter
NCORES = 8
FCH = 6                     # 768 / 128 feature chunks
RANK = 5
KDIM = RANK + 1             # rank rows + ones row carrying the bias
F32 = mybir.dt.float32
BF16 = mybir.dt.bfloat16

N_WARMUP = 30               # dummy PE matmuls to trip the HAM clock un-throttle

A_COLS = FCH * RANK                        # 30
XT_COLS = A_COLS + NPARTS * HID            # 30 + 3072

_CACHE = {}


class _LeanTileContext(tile.TileContext):
    """TileContext with a minimal exit sequence.

    The stock exit emits drain + all-engine barrier + per-sem clears +
    barrier (~2-3us).  The runtime re-initializes semaphore state on every
    NEFF execution (verified empirically: repeated executions of the same
    loaded executable stay bit-correct without the clears), so only the
    drain — which makes the kernel end wait for the output DMAs — is kept.
    """

    def _drain_and_barrier(self, tick_clock, wait_clock):
        drain_inst = self.nc.sync.drain()
        wait_clock.add_sem_waits(
            drain_inst.ins, tile.ScopedClock({None: tick_clock.global_clock})
        )
        popped = self.nc._tile_sem_poison_stack.pop()
        assert popped is self._sem_poison


def _build_program(act=None):
    if act is None:
        act = mybir.ActivationFunctionType.Gelu
    nc = bacc.Bacc(None, target_bir_lowering=False)
    xt = nc.dram_tensor("xt", [128, XT_COLS], BF16, kind="ExternalInput")
    bm = nc.dram_tensor("bm", [KDIM, HID], BF16, kind="ExternalInput")
    outt = nc.dram_tensor("outt", [128, NPARTS * HID], BF16, kind="ExternalOutput")

    with _LeanTileContext(nc) as tc:
        with (
            tc.tile_pool(name="const", bufs=1) as cpool,
            tc.tile_pool(name="xs", bufs=1) as xpool,
            tc.tile_pool(name="work", bufs=2) as wpool,
            tc.tile_pool(name="ps_t3", bufs=2, space="PSUM") as tpool,
            tc.tile_pool(name="ps_o", bufs=2, space="PSUM") as opool,
            tc.tile_pool(name="ps_w", bufs=1, space="PSUM") as wps_pool,
        ):
            # B lands on the scalar-engine HWDGE queue so the sync queue's
            # serial ~600ns-per-DMA issue budget is spent on x alone
            bm_sb = cpool.tile([KDIM, HID], BF16)
            nc.scalar.dma_start(bm_sb[:], bm[:])

            x_sb = xpool.tile([128, XT_COLS], BF16)
            a_view = x_sb[:, 0:A_COLS]

            def xq(q, c=0):
                return x_sb[:, A_COLS + q * HID + c * PSIZE : A_COLS + q * HID + (c + 1) * PSIZE]

            # q0 (split in two so mm1 q0's ~2.2us DMA-completion-sem wait
            # starts from an earlier, smaller first transfer) and q1 go on
            # the sync HWDGE queue; q2/q3 go on the gpsimd SWDGE queue,
            # issuing and wiring in parallel so the back-half quarters'
            # completion sems arrive ~1us earlier
            cuts = [0, A_COLS + HID // 2] + [A_COLS + q * HID for q in range(1, NPARTS + 1)]
            for i, (s, e) in enumerate(zip(cuts[:-1], cuts[1:])):
                dma = nc.sync if i < 3 else nc.gpsimd
                dma.dma_start(x_sb[:, s:e], xt[:, s:e])

            # PE warmup: garbage matmuls so the HAM clock gate opens while
            # the x loads are still in flight.  The gate needs ~3.4us of
            # sustained PE activity.  Memsets go on DVE (idle until the
            # first cast) — on gpsimd they would queue behind the q2/q3
            # load issues and delay the warmup (and so the whole PE chain).
            wsb = cpool.tile([128, 128], BF16)
            nc.vector.memset(wsb[:], 0.0)
            wps = wps_pool.tile([128, 128], F32)
            for _ in range(N_WARMUP):
                nc.tensor.matmul(wps[:], wsb[:], wsb[:], start=True, stop=True)

            # rows 0-4 of t3_sb get the per-quarter TT activations; row 5
            # stays at the memset 1.0 and meets the bias row of bm_sb in mm2
            t3_sb = cpool.tile([128, ROWS], BF16)
            nc.vector.memset(t3_sb[:], 1.0)

            for q in range(NPARTS):
                t3_ps = tpool.tile([RANK, PSIZE], F32, tag="t3_ps")
                for c in range(FCH):
                    nc.tensor.matmul(
                        t3_ps[:],
                        a_view[:, c * RANK : (c + 1) * RANK],
                        xq(q, c),
                        start=(c == 0),
                        stop=(c == FCH - 1),
                    )
                nc.vector.tensor_copy(
                    t3_sb[0:RANK, q * PSIZE : (q + 1) * PSIZE], t3_ps[:]
                )

                # (128,1024) f32 = exactly 2 PSUM banks; cols 0-767 used.
                # start=True on the first matmul touching each bank clears
                # that bank's has_written bits; later ones overwrite their
                # still-clear regions.
                o_ps = opool.tile([128, 1024], F32, tag="o_ps")
                for j in range(FCH):
                    nc.tensor.matmul(
                        o_ps[:, j * PSIZE : (j + 1) * PSIZE],
                        bm_sb[:, j * PSIZE : (j + 1) * PSIZE],
                        t3_sb[0:KDIM, q * PSIZE : (q + 1) * PSIZE],
                        start=(j in (0, 4)),
                        stop=(j in (3, 5)),
                    )
                xq_full = x_sb[:, A_COLS + q * HID : A_COLS + (q + 1) * HID]
                o_sb = wpool.tile([128, HID], BF16, tag="o_sb", bufs=4)
                # one N=768 gelu per quarter straight from PSUM amortizes
                # the ~293ns per-op ACT overhead over the whole quarter
                g_sb = wpool.tile([128, HID], BF16, tag="g_sb", bufs=2)
                nc.scalar.activation(g_sb[:], o_ps[:, 0:HID], act, scale=1.0)
                if q < NPARTS - 1:
                    nc.vector.tensor_add(o_sb[:], g_sb[:], xq_full)
                    dma = nc.sync if q % 2 == 0 else nc.gpsimd
                    dma.dma_start(outt[:, q * HID : (q + 1) * HID], o_sb[:])
                else:
                    # last quarter: add+store in column halves across both
                    # DGE queues so the final ~2.2us store receipt starts
                    # during the second half's add
                    HH = HID // 2
                    for k, dma in ((0, nc.sync), (1, nc.gpsimd)):
                        nc.vector.tensor_add(
                            o_sb[:, k * HH : (k + 1) * HH],
                            g_sb[:, k * HH : (k + 1) * HH],
                            xq_full[:, k * HH : (k + 1) * HH],
                        )
                        dma.dma_start(
                            outt[:, q * HID + k * HH : q * HID + (k + 1) * HH],
                            o_sb[:, k * HH : (k + 1) * HH],
                        )

    nc.finalize()
    return nc


def _get_program():
    if "nc" not in _CACHE:
        _CACHE["nc"] = _build_program()
    return _CACHE["nc"]


def _host_prep(hidden_states, bias, cores):
    """Collapse TT cores to rank-5 factors; pack A + x^T per core in bf16."""
    c0, c1, c2, c3, c4, c5 = [c.astype(np.float64) for c in cores]
    A = np.einsum("iv,vjw,wkx->ijkx", c0[0], c1, c2).reshape(HID, RANK)
    Bm = np.einsum("xpy,yqz,zr->xpqr", c3, c4, c5[:, :, 0]).reshape(RANK, HID)

    a_p = np.ascontiguousarray(
        A.reshape(FCH, 128, RANK).transpose(1, 0, 2).reshape(128, A_COLS)
    ).astype(ml_dtypes.bfloat16)                       # (128, 30)
    bm_p = np.empty((KDIM, HID), dtype=ml_dtypes.bfloat16)
    bm_p[:RANK] = Bm.astype(ml_dtypes.bfloat16)
    bm_p[RANK] = bias.astype(ml_dtypes.bfloat16)       # meets t3_sb's ones row

    xts = []
    for cidx in range(NCORES):
        xct = hidden_states[cidx].T                    # (768, 512) f32
        blocks = [a_p]
        for q in range(NPARTS):
            blocks.append(
                np.ascontiguousarray(xct[:, q * PSIZE : (q + 1) * PSIZE])
                .reshape(FCH, 128, PSIZE)
                .transpose(1, 0, 2)
                .reshape(128, FCH * PSIZE)
                .astype(ml_dtypes.bfloat16)
            )
        xts.append(np.ascontiguousarray(np.concatenate(blocks, axis=1)))
    return xts, bm_p


def _unpack_out(outt_list):
    """outt[p, q*768 + j*128 + r] = out[q*128+r, j*128+p] -> (8, 512, 768)."""
    outs = []
    for outt in outt_list:
        m = np.asarray(outt).reshape(128, NPARTS, FCH, PSIZE)
        o = m.transpose(1, 3, 2, 0).reshape(ROWS, HID)
        outs.append(o)
    return np.stack(outs, axis=0).astype(np.float32)


def run(inputs, trace=False, **spmd_kwargs):
    hidden_states = np.asarray(inputs["hidden_states"], dtype=np.float32)
    bias = np.asarray(inputs["bias"], dtype=np.float32)
    cores = [np.asarray(inputs[f"core{i}"], dtype=np.float32) for i in range(6)]

    xts, bm_p = _host_prep(hidden_states, bias, cores)
    nc = _get_program()
    in_maps = [{"xt": xts[c], "bm": bm_p} for c in range(NCORES)]
    res = run_bass_kernel_spmd(
        nc, in_maps, core_ids=list(range(NCORES)), trace=trace, **spmd_kwargs
    )
    out = _unpack_out([res.results[c]["outt"] for c in range(NCORES)])
    if trace:
        return out, res
    return out


def kernel(**inputs):
    return run(inputs)


# revision 17
# speedup vs baseline: 1.0666x; 1.0236x over previous
# Trainium2 Bass kernel for nn_BertAdapter_SLT_49933289783411
#
# Reference computation:
#   y   = tt_linear(x) + bias          (TT-factorized 768->768 linear)
#   out = x + gelu_exact(y)
#
# Key math: the TT cores with ranks [1,5,5,5,5,5,1] factor the 768x768
# weight as W = A @ B with A:(768,5), B:(5,768).  We precompute A,B on
# host (tiny, exact) and run a rank-5 bottleneck matmul on device.
#
# Sharding: data-parallel over the batch dim (8 batch elements -> 8 cores).
# Each core handles x_c:(512,768).  All I/O is bf16 (halves HBM traffic;
# the 2e-2 rel-err budget dwarfs bf16 rounding).  x is pre-transposed on
# host to x^T (feature-major) so the contraction dim lands on SBUF
# partitions.  The 512 rows are processed as 4 quarters of 128 rows, each
# flowing load -> mm1 -> cast -> mm2 -> gelu -> add -> store so the ACT
# engine (the serial bottleneck: ~2.6us of gelu element work) starts as
# early as possible and every stage pipelines across quarters.
#
# Per quarter q (all operands bf16, PSUM accumulation f32):
#   t3_q   = A^T @ x^T_q            (5,128)   PSUM, accumulate over 6 f-chunks
#   y^T_q  = B6^T @ t36_q           (128,768) K=6: B6 rows 0-4 = B, row 5 =
#                                   bias against an all-ones t3 row 5
#   o^T_q  = x^T_q + gelu(y^T_q)    one N=768 gelu op straight from PSUM
#
# B is shipped compact as (6,768) bf16 (9KB) instead of zero-padded to
# K=128 (196KB).  A (128x30 bf16) rides in the head of the x tensor so the
# sync queue issues exactly one DMA per quarter; B goes on the scalar
# (ACT HWDGE) queue; stores alternate sync/gpsimd queues to split the
# ~600ns per-DMA sequencer issue cost.

import numpy as np
import ml_dtypes

import concourse.bass as bass
import concourse.bacc as bacc
import concourse.mybir as mybir
import concourse.tile as tile
from concourse.bass_utils import run_bass_kernel_spmd

HID = 768
ROWS = 512
NPARTS = 4
PSIZE = ROWS // NPARTS      # 128 rows per quarter
NCORES = 8
FCH = 6                     # 768 / 128 feature chunks
RANK = 5
KDIM = RANK + 1             # rank rows + ones row carrying the bias
F32 = mybir.dt.float32
BF16 = mybir.dt.bfloat16

N_WARMUP = 32               # dummy PE matmuls to trip the HAM clock un-throttle

A_COLS = FCH * RANK                        # 30
XT_COLS = A_COLS + NPARTS * HID            # 30 + 3072

_CACHE = {}


class _LeanTileContext(tile.TileContext):
    """TileContext with a minimal exit sequence.

    The stock exit emits drain + all-engine barrier + per-sem clears +
    barrier (~2-3us).  The runtime re-initializes semaphore state on every
    NEFF execution (verified empirically: repeated executions of the same
    loaded executable stay bit-correct without the clears), so only the
    drain — which makes the kernel end wait for the output DMAs — is kept.
    """

    def _drain_and_barrier(self, tick_clock, wait_clock):
        drain_inst = self.nc.sync.drain()
        wait_clock.add_sem_waits(
            drain_inst.ins, tile.ScopedClock({None: tick_clock.global_clock})
        )
        popped = self.nc._tile_sem_poison_stack.pop()
        assert popped is self._sem_poison


def _build_program(act=None):
    if act is None:
        act = mybir.ActivationFunctionType.Gelu
    nc = bacc.Bacc(None, target_bir_lowering=False)
    xt = nc.dram_tensor("xt", [128, XT_COLS], BF16, kind="ExternalInput")
    bm = nc.dram_tensor("bm", [KDIM, HID], BF16, kind="ExternalInput")
    outt = nc.dram_tensor("outt", [128, NPARTS * HID], BF16, kind="ExternalOutput")

    with _LeanTileContext(nc) as tc:
        with (
            tc.tile_pool(name="const", bufs=1) as cpool,
            tc.tile_pool(name="xs", bufs=1) as xpool,
            tc.tile_pool(name="work", bufs=2) as wpool,
            tc.tile_pool(name="ps_t3", bufs=2, space="PSUM") as tpool,
            tc.tile_pool(name="ps_o", bufs=2, space="PSUM") as opool,
            tc.tile_pool(name="ps_w", bufs=1, space="PSUM") as wps_pool,
        ):
            # B lands on the scalar-engine HWDGE queue so the sync queue's
            # serial ~600ns-per-DMA issue budget is spent on x alone
            bm_sb = cpool.tile([KDIM, HID], BF16)
            nc.scalar.dma_start(bm_sb[:], bm[:])

            x_sb = xpool.tile([128, XT_COLS], BF16)
            a_view = x_sb[:, 0:A_COLS]

            def xq(q, c=0):
                return x_sb[:, A_COLS + q * HID + c * PSIZE : A_COLS + q * HID + (c + 1) * PSIZE]

            # q0 (split in two so mm1 q0's ~2.2us DMA-completion-sem wait
            # starts from an earlier, smaller first transfer) and q1 go on
            # the sync HWDGE queue; q2/q3 go on the gpsimd SWDGE queue,
            # issuing and wiring in parallel so the back-half quarters'
            # completion sems arrive ~1us earlier
            cuts = [0, A_COLS + HID // 2] + [A_COLS + q * HID for q in range(1, NPARTS + 1)]
            for i, (s, e) in enumerate(zip(cuts[:-1], cuts[1:])):
                dma = nc.sync if i < 3 else nc.gpsimd
                dma.dma_start(x_sb[:, s:e], xt[:, s:e])

            # PE warmup: garbage matmuls so the HAM clock gate opens while
            # the x loads are still in flight.  The gate needs ~3.4us of
            # sustained PE activity.  Memsets go on DVE (idle until the
            # first cast) — on gpsimd they would queue behind the q2/q3
            # load issues and delay the warmup (and so the whole PE chain).
            wsb = cpool.tile([128, 128], BF16)
            nc.vector.memset(wsb[:], 0.0)
            wps = wps_pool.tile([128, 128], F32)
            for _ in range(N_WARMUP):
                nc.tensor.matmul(wps[:], wsb[:], wsb[:], start=True, stop=True)

            # rows 0-4 of t3_sb get the per-quarter TT activations; row 5
            # stays at the memset 1.0 and meets the bias row of bm_sb in mm2
            t3_sb = cpool.tile([128, ROWS], BF16)
            nc.vector.memset(t3_sb[:], 1.0)

            for q in range(NPARTS):
                t3_ps = tpool.tile([RANK, PSIZE], F32, tag="t3_ps")
                for c in range(FCH):
                    nc.tensor.matmul(
                        t3_ps[:],
                        a_view[:, c * RANK : (c + 1) * RANK],
                        xq(q, c),
                        start=(c == 0),
                        stop=(c == FCH - 1),
                    )
                nc.vector.tensor_copy(
                    t3_sb[0:RANK, q * PSIZE : (q + 1) * PSIZE], t3_ps[:]
                )

                # (128,1024) f32 = exactly 2 PSUM banks; cols 0-767 used.
                # start=True on the first matmul touching each bank clears
                # that bank's has_written bits; later ones overwrite their
                # still-clear regions.
                o_ps = opool.tile([128, 1024], F32, tag="o_ps")
                for j in range(FCH):
                    nc.tensor.matmul(
                        o_ps[:, j * PSIZE : (j + 1) * PSIZE],
                        bm_sb[:, j * PSIZE : (j + 1) * PSIZE],
                        t3_sb[0:KDIM, q * PSIZE : (q + 1) * PSIZE],
                        start=(j in (0, 4)),
                        stop=(j in (3, 5)),
                    )
                xq_full = x_sb[:, A_COLS + q * HID : A_COLS + (q + 1) * HID]
                o_sb = wpool.tile([128, HID], BF16, tag="o_sb", bufs=4)
                # one N=768 gelu per quarter straight from PSUM amortizes
                # the ~293ns per-op ACT overhead over the whole quarter
                g_sb = wpool.tile([128, HID], BF16, tag="g_sb", bufs=2)
                nc.scalar.activation(g_sb[:], o_ps[:, 0:HID], act, scale=1.0)
                if q < NPARTS - 1:
                    nc.vector.tensor_add(o_sb[:], g_sb[:], xq_full)
                    dma = nc.sync if q % 2 == 0 else nc.gpsimd
                    dma.dma_start(outt[:, q * HID : (q + 1) * HID], o_sb[:])
                else:
                    # last quarter: add+store in column halves across both
                    # DGE queues so the final ~2.2us store receipt starts
                    # during the second half's add
                    HH = HID // 2
                    for k, dma in ((0, nc.sync), (1, nc.gpsimd)):
                        nc.vector.tensor_add(
                            o_sb[:, k * HH : (k + 1) * HH],
                            g_sb[:, k * HH : (k + 1) * HH],
                            xq_full[:, k * HH : (k + 1) * HH],
                        )
                        dma.dma_start(
                            outt[:, q * HID + k * HH : q * HID + (k + 1) * HH],
                            o_sb[:, k * HH : (k + 1) * HH],
                        )

    nc.finalize()
    return nc


def _get_program():
    if "nc" not in _CACHE:
        _CACHE["nc"] = _build_program()
    return _CACHE["nc"]


def _host_prep(hidden_states, bias, cores):
    """Collapse TT cores to rank-5 factors; pack A + x^T per core in bf16."""
    c0, c1, c2, c3, c4, c5 = [c.astype(np.float64) for c in cores]
    A = np.einsum("iv,vjw,wkx->ijkx", c0[0], c1, c2).reshape(HID, RANK)
    Bm = np.einsum("xpy,yqz,zr->xpqr", c3, c4, c5[:, :, 0]).reshape(RANK, HID)

    a_p = np.ascontiguousarray(
        A.reshape(FCH, 128, RANK).transpose(1, 0, 2).reshape(128, A_COLS)
    ).astype(ml_dtypes.bfloat16)                       # (128, 30)
    bm_p = np.empty((KDIM, HID), dtype=ml_dtypes.bfloat16)
    bm_p[:RANK] = Bm.astype(ml_dtypes.bfloat16)
    bm_p[RANK] = bias.astype(ml_dtypes.bfloat16)       # meets t3_sb's ones row

    xts = []
    for cidx in range(NCORES):
        xct = hidden_states[cidx].T                    # (768, 512) f32
        blocks = [a_p]
        for q in range(NPARTS):
            blocks.append(
                np.ascontiguousarray(xct[:, q * PSIZE : (q + 1) * PSIZE])
                .reshape(FCH, 128, PSIZE)
                .transpose(1, 0, 2)
                .reshape(128, FCH * PSIZE)
                .astype(ml_dtypes.bfloat16)
            )
        xts.append(np.ascontiguousarray(np.concatenate(blocks, axis=1)))
    return xts, bm_p


def _unpack_out(outt_list):
    """outt[p, q*768 + j*128 + r] = out[q*128+r, j*128+p] -> (8, 512, 768)."""
    outs = []
    for outt in outt_list:
        m = np.asarray(outt).reshape(128, NPARTS, FCH, PSIZE)
        o = m.transpose(1, 3, 2, 0).reshape(ROWS, HID)
        outs.append(o)
    return np.stack(outs, axis=0).astype(np.float32)


def run(inputs, trace=False, **spmd_kwargs):
    hidden_states = np.asarray(inputs["hidden_states"], dtype=np.float32)
    bias = np.asarray(inputs["bias"], dtype=np.float32)
    cores = [np.asarray(inputs[f"core{i}"], dtype=np.float32) for i in range(6)]

    xts, bm_p = _host_prep(hidden_states, bias, cores)
    nc = _get_program()
    in_maps = [{"xt": xts[c], "bm": bm_p} for c in range(NCORES)]
    res = run_bass_kernel_spmd(
        nc, in_maps, core_ids=list(range(NCORES)), trace=trace, **spmd_kwargs
    )
    out = _unpack_out([res.results[c]["outt"] for c in range(NCORES)])
    if trace:
        return out, res
    return out


def kernel(**inputs):
    return run(inputs)
